# revision 10
# baseline (speedup 1.0000x reference)
"""Cross-attention Bass kernel for Trainium2, data-parallel over batch.

Problem (hardcoded): b=8, c=256, h=w=64 (n=4096).
  q = Wq@hsv + bq; k = Wk@rgb + bk; v = Wv@rgb + bv   (1x1 convs, [c, n])
  attn = softmax_j(q_i . k_j / sqrt(c)); out[c,i] = sum_j v[c,j] attn[i,j]

Per-core design (one batch per NeuronCore, 8 cores):
  - Host pre-transposes weights (WqT/WkT/WvT = W.T) and folds the 1/sqrt(c)
    scale into WqT and bq, so no on-device transposes or scaling are needed.
  - S^T layout: S^T[j, i] tiles computed as lhsT=K-chunk, rhs=Q-chunk, so the
    softmax axis j lands on PSUM partitions and P^T = exp(S^T) is directly the
    lhsT the PV matmul needs. Scores are in [-0.7, 0.7] (tiny weights), so
    exp without max-subtraction is exact softmax.
  - V^T carries an extra ones column: out^T[i, 0:256] accumulates P@V^T and
    out^T[i, 256] accumulates the softmax denominator in the same matmuls.
  - bv is folded in after normalization (sum_j attn = 1 passes bias through).
  - fp32r (relaxed fp32) matmuls: full 1 cyc/row speed at N>=256. The whole
    matmul data path (DRAM inputs, staging, Q/K/V, P^T) is typed float32r
    because the BIR verifier requires producers of fp32r matmul operands to
    emit fp32r.
  - Software pipeline: S/exp of i-tile t+1 interleaved with PV of i-tile t so
    the ScalarE exp stream hides entirely under TensorE work.
"""

import numpy as np

B, C, H, W = 8, 256, 64, 64
N = H * W          # 4096
CK = C // 128      # 2 contraction/channel chunks
NJ = N // 128      # 32 key blocks
NT = N // 512      # 8 query tiles of 512
NSUB = 4           # 128-wide query sub-blocks per query tile

_CACHE = {}

# ones column for the rowsum trick + zero pad to an even fp32r moving dim
_VONE = np.concatenate(
    [np.ones((128, NJ, 1), np.float32), np.zeros((128, NJ, 1), np.float32)], axis=2
)


def _build():
    import concourse.tile as tile
    from concourse import bacc, mybir
    from concourse.masks import make_identity
    from contextlib import ExitStack

    f32 = mybir.dt.float32
    f32r = mybir.dt.float32r

    nc = bacc.Bacc(None, target_bir_lowering=False)

    hsv = nc.dram_tensor("hsv", [C, N], f32r, kind="ExternalInput")
    rgb = nc.dram_tensor("rgb", [C, N], f32r, kind="ExternalInput")
    wqT = nc.dram_tensor("wqT", [C, C], f32r, kind="ExternalInput")
    wkT = nc.dram_tensor("wkT", [C, C], f32r, kind="ExternalInput")
    wvT = nc.dram_tensor("wvT", [C, C], f32r, kind="ExternalInput")
    bqd = nc.dram_tensor("bq", [C, 1], f32, kind="ExternalInput")
    bkd = nc.dram_tensor("bk", [C, 1], f32, kind="ExternalInput")
    bvd = nc.dram_tensor("bv", [C, 1], f32, kind="ExternalInput")
    vone = nc.dram_tensor("vone", [128, NJ, 2], f32r, kind="ExternalInput")
    out = nc.dram_tensor("out", [C, N], f32, kind="ExternalOutput")

    with tile.TileContext(nc) as tc, ExitStack() as ctx:
        consts = ctx.enter_context(tc.tile_pool(name="consts", bufs=1))
        big = ctx.enter_context(tc.tile_pool(name="big", bufs=1))

        wq_sb = consts.tile([128, CK, C], f32r, name="wq_sb")
        wk_sb = consts.tile([128, CK, C], f32r, name="wk_sb")
        wv_sb = consts.tile([128, CK, C], f32r, name="wv_sb")
        bq_sb = consts.tile([128, CK, 1], f32, name="bq_sb")
        bk_sb = consts.tile([128, CK, 1], f32, name="bk_sb")
        bv_sb = consts.tile([128, CK, 1], f32, name="bv_sb")
        ident = consts.tile([128, 128], f32, name="ident")

        nc.sync.dma_start(out=wq_sb[:], in_=wqT.rearrange("(k p) m -> p k m", p=128))
        nc.sync.dma_start(out=wk_sb[:], in_=wkT.rearrange("(k p) m -> p k m", p=128))
        nc.sync.dma_start(out=wv_sb[:], in_=wvT.rearrange("(k p) m -> p k m", p=128))
        nc.sync.dma_start(out=bq_sb[:], in_=bqd.rearrange("(k p) o -> p k o", p=128))
        nc.sync.dma_start(out=bk_sb[:], in_=bkd.rearrange("(k p) o -> p k o", p=128))
        nc.sync.dma_start(out=bv_sb[:], in_=bvd.rearrange("(k p) o -> p k o", p=128))
        make_identity(nc, ident)

        # Q, K in [c-chunk-on-partition, n-free] layout; V^T in
        # [n-block-on-partition, c-free] layout with a trailing ones column.
        q_sb = big.tile([128, CK, N], f32r, name="q_sb")
        k_sb = big.tile([128, CK, N], f32r, name="k_sb")
        v_sb = big.tile([128, NJ, C + 2], f32r, name="v_sb")

        hsv_r = hsv.rearrange("(k p) n -> p k n", p=128)
        rgb_r = rgb.rearrange("(k p) n -> p k n", p=128)

        with (
            tc.tile_pool(name="io", bufs=4) as io,
            tc.tile_pool(name="ppsum", bufs=4, space="PSUM") as pp,
        ):
            nc.sync.dma_start(out=v_sb[:, :, C : C + 2], in_=vone[:])
            for t in range(NT):
                xh = io.tile([128, CK, 512], f32r, name="xh", tag="x")
                nc.sync.dma_start(out=xh[:], in_=hsv_r[:, :, t * 512 : (t + 1) * 512])
                for ci in range(CK):
                    ps = pp.tile([128, 512], f32, name="ps_q", tag="pp")
                    for k in range(CK):
                        nc.tensor.matmul(
                            ps,
                            lhsT=wq_sb[:, k, ci * 128 : (ci + 1) * 128],
                            rhs=xh[:, k, :],
                            start=(k == 0),
                            stop=(k == CK - 1),
                        )
                    nc.vector.tensor_scalar_add(
                        q_sb[:, ci, t * 512 : (t + 1) * 512], ps, bq_sb[:, ci, :]
                    )

                xr = io.tile([128, CK, 512], f32r, name="xr", tag="x")
                nc.sync.dma_start(out=xr[:], in_=rgb_r[:, :, t * 512 : (t + 1) * 512])
                for ci in range(CK):
                    ps = pp.tile([128, 512], f32, name="ps_k", tag="pp")
                    for k in range(CK):
                        nc.tensor.matmul(
                            ps,
                            lhsT=wk_sb[:, k, ci * 128 : (ci + 1) * 128],
                            rhs=xr[:, k, :],
                            start=(k == 0),
                            stop=(k == CK - 1),
                        )
                    nc.vector.tensor_scalar_add(
                        k_sb[:, ci, t * 512 : (t + 1) * 512], ps, bk_sb[:, ci, :]
                    )

                for jj in range(4):
                    j = t * 4 + jj
                    ps = pp.tile([128, C], f32, name="ps_v", tag="pp")
                    for k in range(CK):
                        nc.tensor.matmul(
                            ps,
                            lhsT=xr[:, k, jj * 128 : (jj + 1) * 128],
                            rhs=wv_sb[:, k, :],
                            start=(k == 0),
                            stop=(k == CK - 1),
                        )
                    nc.vector.tensor_copy(v_sb[:, j, 0:C], ps)

        pt_pool = ctx.enter_context(tc.tile_pool(name="pt", bufs=34))
        spool = ctx.enter_context(tc.tile_pool(name="spsum", bufs=2, space="PSUM"))
        # bufs applies per-tag: 4 tags (po0..po3) x 1 buf = 4 PSUM banks.
        opool = ctx.enter_context(tc.tile_pool(name="opsum", bufs=1, space="PSUM"))
        tpool = ctx.enter_context(tc.tile_pool(name="tpsum", bufs=2, space="PSUM"))
        small = ctx.enter_context(tc.tile_pool(name="small", bufs=6))
        ostage = ctx.enter_context(tc.tile_pool(name="ostage", bufs=4))

        def emit_s(it, j):
            """S^T[j-block, i-tile] = K_j^T Q_i, then P^T = exp(S^T) to SBUF."""
            ps = spool.tile([128, 512], f32, name="ps_s", tag="s")
            for k in range(CK):
                nc.tensor.matmul(
                    ps,
                    lhsT=k_sb[:, k, j * 128 : (j + 1) * 128],
                    rhs=q_sb[:, k, it * 512 : (it + 1) * 512],
                    start=(k == 0),
                    stop=(k == CK - 1),
                )
            pt = pt_pool.tile([128, 512], f32r, name="pt", tag="pt")
            nc.scalar.activation(pt, ps, mybir.ActivationFunctionType.Exp)
            return pt

        cur = [emit_s(0, j) for j in range(NJ)]
        for it in range(NT):
            po = [
                opool.tile([128, C + 2], f32, name=f"po{isub}", tag=f"po{isub}")
                for isub in range(NSUB)
            ]
            nxt = [None] * NJ
            for j in range(NJ):
                for isub in range(NSUB):
                    nc.tensor.matmul(
                        po[isub],
                        lhsT=cur[j][:, isub * 128 : (isub + 1) * 128],
                        rhs=v_sb[:, j, :],
                        start=(j == 0),
                        stop=(j == NJ - 1),
                    )
                if it + 1 < NT:
                    nxt[j] = emit_s(it + 1, j)
            for isub in range(NSUB):
                rec = small.tile([128, 1], f32, name="rec", tag="rec")
                nc.vector.reciprocal(rec, po[isub][:, C : C + 1])
                ot = small.tile([128, C], f32, name="ot", tag="ot")
                nc.vector.tensor_scalar_mul(ot, po[isub][:, 0:C], rec)
                for ci in range(CK):
                    tp = tpool.tile([128, 128], f32, name="tp", tag="tp")
                    nc.tensor.transpose(tp, ot[:, ci * 128 : (ci + 1) * 128], ident)
                    ob = ostage.tile([128, 128], f32, name="ob", tag="ob")
                    nc.vector.tensor_scalar_add(ob, tp, bv_sb[:, ci, :])
                    i0 = it * 512 + isub * 128
                    nc.sync.dma_start(
                        out=out[ci * 128 : (ci + 1) * 128, i0 : i0 + 128], in_=ob
                    )
            cur = nxt

    nc.compile()
    return nc


def _get_nc():
    if "nc" not in _CACHE:
        _CACHE["nc"] = _build()
    return _CACHE["nc"]


def kernel(rgb_feat, hsv_feat, Wq, bq, Wk, bk, Wv, bv, _debug=None):
    from concourse.bass_utils import run_bass_kernel_spmd

    rgb_feat = np.ascontiguousarray(np.asarray(rgb_feat, dtype=np.float32))
    hsv_feat = np.ascontiguousarray(np.asarray(hsv_feat, dtype=np.float32))
    scale = np.float32(1.0) / np.sqrt(np.float32(C))
    wqT = np.ascontiguousarray(np.asarray(Wq, np.float32).T * scale)
    wkT = np.ascontiguousarray(np.asarray(Wk, np.float32).T)
    wvT = np.ascontiguousarray(np.asarray(Wv, np.float32).T)
    bq_ = np.ascontiguousarray((np.asarray(bq, np.float32) * scale).reshape(C, 1))
    bk_ = np.ascontiguousarray(np.asarray(bk, np.float32).reshape(C, 1))
    bv_ = np.ascontiguousarray(np.asarray(bv, np.float32).reshape(C, 1))

    in_maps = []
    for bi in range(B):
        in_maps.append(
            {
                "hsv": np.ascontiguousarray(hsv_feat[bi].reshape(C, N)),
                "rgb": np.ascontiguousarray(rgb_feat[bi].reshape(C, N)),
                "wqT": wqT,
                "wkT": wkT,
                "wvT": wvT,
                "bq": bq_,
                "bk": bk_,
                "bv": bv_,
                "vone": _VONE,
            }
        )

    nc = _get_nc()
    kwargs = dict(_debug or {})
    kwargs.pop("result", None)
    res = run_bass_kernel_spmd(nc, in_maps, core_ids=list(range(B)), **kwargs)
    if _debug is not None:
        _debug["result"] = res
    outs = [res.results[bi]["out"].reshape(C, H, W) for bi in range(B)]
    return np.stack(outs, axis=0).astype(np.float32)


# revision 12
# speedup vs baseline: 1.0173x; 1.0173x over previous
"""Cross-attention Bass kernel for Trainium2, data-parallel over batch.

Problem (hardcoded): b=8, c=256, h=w=64 (n=4096).
  q = Wq@hsv + bq; k = Wk@rgb + bk; v = Wv@rgb + bv   (1x1 convs, [c, n])
  attn = softmax_j(q_i . k_j / sqrt(c)); out[c,i] = sum_j v[c,j] attn[i,j]

Per-core design (one batch per NeuronCore, 8 cores):
  - Host pre-transposes weights (WqT/WkT/WvT = W.T) and folds the 1/sqrt(c)
    scale into WqT and bq, so no on-device transposes or scaling are needed.
  - S^T layout: S^T[j, i] tiles computed as lhsT=K-chunk, rhs=Q-chunk, so the
    softmax axis j lands on PSUM partitions and P^T = exp(S^T) is directly the
    lhsT the PV matmul needs. Scores are in [-0.7, 0.7] (tiny weights), so
    exp without max-subtraction is exact softmax.
  - V^T carries an extra ones column: out^T[i, 0:256] accumulates P@V^T and
    out^T[i, 256] accumulates the softmax denominator in the same matmuls.
  - bv is folded in after normalization (sum_j attn = 1 passes bias through).
  - fp32r (relaxed fp32) matmuls: full 1 cyc/row speed at N>=256. The whole
    matmul data path (DRAM inputs, staging, Q/K/V, P^T) is typed float32r
    because the BIR verifier requires producers of fp32r matmul operands to
    emit fp32r.
  - Software pipeline: S/exp of i-tile t+1 interleaved with PV of i-tile t so
    the ScalarE exp stream hides entirely under TensorE work.
"""

import numpy as np

B, C, H, W = 8, 256, 64, 64
N = H * W          # 4096
CK = C // 128      # 2 contraction/channel chunks
NJ = N // 128      # 32 key blocks
NT = N // 512      # 8 query tiles of 512
NSUB = 4           # 128-wide query sub-blocks per query tile

_CACHE = {}

# ones column for the rowsum trick + zero pad to an even fp32r moving dim
_VONE = np.concatenate(
    [np.ones((128, NJ, 1), np.float16), np.zeros((128, NJ, 1), np.float16)], axis=2
)


def _build():
    import concourse.tile as tile
    from concourse import bacc, mybir
    from concourse.masks import make_identity
    from contextlib import ExitStack

    f32 = mybir.dt.float32
    f32r = mybir.dt.float32r
    f16 = mybir.dt.float16

    nc = bacc.Bacc(None, target_bir_lowering=False)

    hsv = nc.dram_tensor("hsv", [C, N], f32r, kind="ExternalInput")
    rgb = nc.dram_tensor("rgb", [C, N], f32r, kind="ExternalInput")
    wqT = nc.dram_tensor("wqT", [C, C], f32r, kind="ExternalInput")
    wkT = nc.dram_tensor("wkT", [C, C], f32r, kind="ExternalInput")
    wvT = nc.dram_tensor("wvT", [C, C], f32r, kind="ExternalInput")
    bqd = nc.dram_tensor("bq", [C, 1], f32, kind="ExternalInput")
    bkd = nc.dram_tensor("bk", [C, 1], f32, kind="ExternalInput")
    bvd = nc.dram_tensor("bv", [C, 1], f32, kind="ExternalInput")
    vone = nc.dram_tensor("vone", [128, NJ, 2], f16, kind="ExternalInput")
    out = nc.dram_tensor("out", [C, N], f32, kind="ExternalOutput")

    with tile.TileContext(nc) as tc, ExitStack() as ctx:
        consts = ctx.enter_context(tc.tile_pool(name="consts", bufs=1))
        big = ctx.enter_context(tc.tile_pool(name="big", bufs=1))

        wq_sb = consts.tile([128, CK, C], f32r, name="wq_sb")
        wk_sb = consts.tile([128, CK, C], f32r, name="wk_sb")
        wv_sb = consts.tile([128, CK, C], f32r, name="wv_sb")
        bq_sb = consts.tile([128, CK, 1], f32, name="bq_sb")
        bk_sb = consts.tile([128, CK, 1], f32, name="bk_sb")
        bv_sb = consts.tile([128, CK, 1], f32, name="bv_sb")
        ident = consts.tile([128, 128], f32, name="ident")

        nc.sync.dma_start(out=wq_sb[:], in_=wqT.rearrange("(k p) m -> p k m", p=128))
        nc.sync.dma_start(out=bq_sb[:], in_=bqd.rearrange("(k p) o -> p k o", p=128))

        # Q, K in [c-chunk-on-partition, n-free] layout; V^T in
        # [n-block-on-partition, c-free] layout with a trailing ones column.
        q_sb = big.tile([128, CK, N], f32r, name="q_sb")
        k_sb = big.tile([128, CK, N], f32r, name="k_sb")
        v_sb = big.tile([128, NJ, C + 2], f16, name="v_sb")

        hsv_r = hsv.rearrange("(k p) n -> p k n", p=128)
        rgb_r = rgb.rearrange("(k p) n -> p k n", p=128)

        with (
            tc.tile_pool(name="io", bufs=4) as io,
            tc.tile_pool(name="ppsum", bufs=4, space="PSUM") as pp,
        ):
            for t in range(NT):
                xh = io.tile([128, CK, 512], f32r, name="xh", tag="x")
                nc.sync.dma_start(out=xh[:], in_=hsv_r[:, :, t * 512 : (t + 1) * 512])
                if t == 0:
                    # deferred const loads: queued behind the tile the first
                    # matmul needs, but well before their own first use
                    nc.sync.dma_start(
                        out=wk_sb[:], in_=wkT.rearrange("(k p) m -> p k m", p=128)
                    )
                    nc.sync.dma_start(
                        out=bk_sb[:], in_=bkd.rearrange("(k p) o -> p k o", p=128)
                    )
                    nc.sync.dma_start(
                        out=wv_sb[:], in_=wvT.rearrange("(k p) m -> p k m", p=128)
                    )
                    nc.sync.dma_start(
                        out=bv_sb[:], in_=bvd.rearrange("(k p) o -> p k o", p=128)
                    )
                    make_identity(nc, ident)
                    nc.sync.dma_start(out=v_sb[:, :, C : C + 2], in_=vone[:])
                for ci in range(CK):
                    ps = pp.tile([128, 512], f32, name="ps_q", tag="pp")
                    for k in range(CK):
                        nc.tensor.matmul(
                            ps,
                            lhsT=wq_sb[:, k, ci * 128 : (ci + 1) * 128],
                            rhs=xh[:, k, :],
                            start=(k == 0),
                            stop=(k == CK - 1),
                        )
                    nc.vector.tensor_scalar_add(
                        q_sb[:, ci, t * 512 : (t + 1) * 512], ps, bq_sb[:, ci, :]
                    )

                xr = io.tile([128, CK, 512], f32r, name="xr", tag="x")
                nc.sync.dma_start(out=xr[:], in_=rgb_r[:, :, t * 512 : (t + 1) * 512])
                for ci in range(CK):
                    ps = pp.tile([128, 512], f32, name="ps_k", tag="pp")
                    for k in range(CK):
                        nc.tensor.matmul(
                            ps,
                            lhsT=wk_sb[:, k, ci * 128 : (ci + 1) * 128],
                            rhs=xr[:, k, :],
                            start=(k == 0),
                            stop=(k == CK - 1),
                        )
                    nc.vector.tensor_scalar_add(
                        k_sb[:, ci, t * 512 : (t + 1) * 512], ps, bk_sb[:, ci, :]
                    )

                for jj in range(4):
                    j = t * 4 + jj
                    ps = pp.tile([128, C], f32, name="ps_v", tag="pp")
                    for k in range(CK):
                        nc.tensor.matmul(
                            ps,
                            lhsT=xr[:, k, jj * 128 : (jj + 1) * 128],
                            rhs=wv_sb[:, k, :],
                            start=(k == 0),
                            stop=(k == CK - 1),
                        )
                    nc.vector.tensor_copy(v_sb[:, j, 0:C], ps)

        pt_pool = ctx.enter_context(tc.tile_pool(name="pt", bufs=40))
        spool = ctx.enter_context(tc.tile_pool(name="spsum", bufs=2, space="PSUM"))
        # bufs applies per-tag: 4 tags (po0..po3) x 1 buf = 4 PSUM banks.
        opool = ctx.enter_context(tc.tile_pool(name="opsum", bufs=1, space="PSUM"))
        tpool = ctx.enter_context(tc.tile_pool(name="tpsum", bufs=2, space="PSUM"))
        small = ctx.enter_context(tc.tile_pool(name="small", bufs=6))
        ostage = ctx.enter_context(tc.tile_pool(name="ostage", bufs=4))

        def emit_s(it, j):
            """S^T[j-block, i-tile] = K_j^T Q_i, then P^T = exp(S^T) to SBUF."""
            ps = spool.tile([128, 512], f32, name="ps_s", tag="s")
            for k in range(CK):
                nc.tensor.matmul(
                    ps,
                    lhsT=k_sb[:, k, j * 128 : (j + 1) * 128],
                    rhs=q_sb[:, k, it * 512 : (it + 1) * 512],
                    start=(k == 0),
                    stop=(k == CK - 1),
                )
            pt = pt_pool.tile([128, 512], f16, name="pt", tag="pt")
            nc.scalar.activation(pt, ps, mybir.ActivationFunctionType.Exp)
            return pt

        cur = [emit_s(0, j) for j in range(NJ)]
        for it in range(NT):
            po = [
                opool.tile([128, C + 2], f32, name=f"po{isub}", tag=f"po{isub}")
                for isub in range(NSUB)
            ]
            nxt = [None] * NJ
            for j in range(NJ):
                for isub in range(NSUB):
                    nc.tensor.matmul(
                        po[isub],
                        lhsT=cur[j][:, isub * 128 : (isub + 1) * 128],
                        rhs=v_sb[:, j, :],
                        start=(j == 0),
                        stop=(j == NJ - 1),
                    )
                if it + 1 < NT:
                    nxt[j] = emit_s(it + 1, j)
            for isub in range(NSUB):
                rec = small.tile([128, 1], f32, name="rec", tag="rec")
                nc.vector.reciprocal(rec, po[isub][:, C : C + 1])
                ot = small.tile([128, C], f32, name="ot", tag="ot")
                nc.vector.tensor_scalar_mul(ot, po[isub][:, 0:C], rec)
                for ci in range(CK):
                    tp = tpool.tile([128, 128], f32, name="tp", tag="tp")
                    nc.tensor.transpose(tp, ot[:, ci * 128 : (ci + 1) * 128], ident)
                    ob = ostage.tile([128, 128], f32, name="ob", tag="ob")
                    nc.vector.tensor_scalar_add(ob, tp, bv_sb[:, ci, :])
                    i0 = it * 512 + isub * 128
                    nc.sync.dma_start(
                        out=out[ci * 128 : (ci + 1) * 128, i0 : i0 + 128], in_=ob
                    )
            cur = nxt

    nc.compile()
    return nc


def _get_nc():
    if "nc" not in _CACHE:
        _CACHE["nc"] = _build()
    return _CACHE["nc"]


def kernel(rgb_feat, hsv_feat, Wq, bq, Wk, bk, Wv, bv, _debug=None):
    from concourse.bass_utils import run_bass_kernel_spmd

    rgb_feat = np.ascontiguousarray(np.asarray(rgb_feat, dtype=np.float32))
    hsv_feat = np.ascontiguousarray(np.asarray(hsv_feat, dtype=np.float32))
    scale = np.float32(1.0) / np.sqrt(np.float32(C))
    wqT = np.ascontiguousarray(np.asarray(Wq, np.float32).T * scale)
    wkT = np.ascontiguousarray(np.asarray(Wk, np.float32).T)
    wvT = np.ascontiguousarray(np.asarray(Wv, np.float32).T)
    bq_ = np.ascontiguousarray((np.asarray(bq, np.float32) * scale).reshape(C, 1))
    bk_ = np.ascontiguousarray(np.asarray(bk, np.float32).reshape(C, 1))
    bv_ = np.ascontiguousarray(np.asarray(bv, np.float32).reshape(C, 1))

    in_maps = []
    for bi in range(B):
        in_maps.append(
            {
                "hsv": np.ascontiguousarray(hsv_feat[bi].reshape(C, N)),
                "rgb": np.ascontiguousarray(rgb_feat[bi].reshape(C, N)),
                "wqT": wqT,
                "wkT": wkT,
                "wvT": wvT,
                "bq": bq_,
                "bk": bk_,
                "bv": bv_,
                "vone": _VONE,
            }
        )

    nc = _get_nc()
    kwargs = dict(_debug or {})
    kwargs.pop("result", None)
    res = run_bass_kernel_spmd(nc, in_maps, core_ids=list(range(B)), **kwargs)
    if _debug is not None:
        _debug["result"] = res
    outs = [res.results[bi]["out"].reshape(C, H, W) for bi in range(B)]
    return np.stack(outs, axis=0).astype(np.float32)


# revision 13
# speedup vs baseline: 1.0750x; 1.0566x over previous
"""Cross-attention Bass kernel for Trainium2, data-parallel over batch.

Problem (hardcoded): b=8, c=256, h=w=64 (n=4096).
  q = Wq@hsv + bq; k = Wk@rgb + bk; v = Wv@rgb + bv   (1x1 convs, [c, n])
  attn = softmax_j(q_i . k_j / sqrt(c)); out[c,i] = sum_j v[c,j] attn[i,j]

Per-core design (one batch per NeuronCore, 8 cores):
  - Host pre-transposes weights (WqT/WkT/WvT = W.T) and folds the 1/sqrt(c)
    scale into WqT and bq, so no on-device transposes or scaling are needed.
  - S^T layout: S^T[j, i] tiles computed as lhsT=K-chunk, rhs=Q-chunk, so the
    softmax axis j lands on PSUM partitions and P^T = exp(S^T) is directly the
    lhsT the PV matmul needs. Scores are in [-0.7, 0.7] (tiny weights), so
    exp without max-subtraction is exact softmax.
  - V^T carries an extra ones column: out^T[i, 0:256] accumulates P@V^T and
    out^T[i, 256] accumulates the softmax denominator in the same matmuls.
  - bv is folded in after normalization (sum_j attn = 1 passes bias through).
  - fp32r (relaxed fp32) matmuls: full 1 cyc/row speed at N>=256. The whole
    matmul data path (DRAM inputs, staging, Q/K/V, P^T) is typed float32r
    because the BIR verifier requires producers of fp32r matmul operands to
    emit fp32r.
  - Software pipeline: S/exp of i-tile t+1 interleaved with PV of i-tile t so
    the ScalarE exp stream hides entirely under TensorE work.
"""

import numpy as np

B, C, H, W = 8, 256, 64, 64
N = H * W          # 4096
CK = C // 128      # 2 contraction/channel chunks
NJ = N // 128      # 32 key blocks
NT = N // 512      # 8 query tiles of 512
NSUB = 4           # 128-wide query sub-blocks per query tile

_CACHE = {}

# ones column for the rowsum trick + zero pad to an even fp32r moving dim
_VONE = np.concatenate(
    [np.ones((128, NJ, 1), np.float16), np.zeros((128, NJ, 1), np.float16)], axis=2
)


def _build():
    import concourse.tile as tile
    from concourse import bacc, mybir
    from concourse.masks import make_identity
    from contextlib import ExitStack

    f32 = mybir.dt.float32
    f32r = mybir.dt.float32r
    f16 = mybir.dt.float16

    nc = bacc.Bacc(None, target_bir_lowering=False)

    hsv = nc.dram_tensor("hsv", [C, N], f16, kind="ExternalInput")
    rgb = nc.dram_tensor("rgb", [C, N], f16, kind="ExternalInput")
    wqT = nc.dram_tensor("wqT", [C, C], f16, kind="ExternalInput")
    wkT = nc.dram_tensor("wkT", [C, C], f16, kind="ExternalInput")
    wvT = nc.dram_tensor("wvT", [C, C], f16, kind="ExternalInput")
    bqd = nc.dram_tensor("bq", [C, 1], f32, kind="ExternalInput")
    bkd = nc.dram_tensor("bk", [C, 1], f32, kind="ExternalInput")
    bvd = nc.dram_tensor("bv", [C, 1], f32, kind="ExternalInput")
    vone = nc.dram_tensor("vone", [128, NJ, 2], f16, kind="ExternalInput")
    out = nc.dram_tensor("out", [C, N], f32, kind="ExternalOutput")

    with tile.TileContext(nc) as tc, ExitStack() as ctx:
        consts = ctx.enter_context(tc.tile_pool(name="consts", bufs=1))
        big = ctx.enter_context(tc.tile_pool(name="big", bufs=1))

        wq_sb = consts.tile([128, CK, C], f16, name="wq_sb")
        wk_sb = consts.tile([128, CK, C], f16, name="wk_sb")
        wv_sb = consts.tile([128, CK, C], f16, name="wv_sb")
        bq_sb = consts.tile([128, CK, 1], f32, name="bq_sb")
        bk_sb = consts.tile([128, CK, 1], f32, name="bk_sb")
        bv_sb = consts.tile([128, CK, 1], f32, name="bv_sb")
        ident = consts.tile([128, 128], f32, name="ident")

        nc.sync.dma_start(out=wq_sb[:], in_=wqT.rearrange("(k p) m -> p k m", p=128))
        nc.sync.dma_start(out=bq_sb[:], in_=bqd.rearrange("(k p) o -> p k o", p=128))

        # Q, K in [c-chunk-on-partition, n-free] layout; V^T in
        # [n-block-on-partition, c-free] layout with a trailing ones column.
        q_sb = big.tile([128, CK, N], f16, name="q_sb")
        k_sb = big.tile([128, CK, N], f16, name="k_sb")
        v_sb = big.tile([128, NJ, C + 2], f16, name="v_sb")

        hsv_r = hsv.rearrange("(k p) n -> p k n", p=128)
        rgb_r = rgb.rearrange("(k p) n -> p k n", p=128)

        with (
            tc.tile_pool(name="io", bufs=4) as io,
            tc.tile_pool(name="ppsum", bufs=4, space="PSUM") as pp,
        ):
            for t in range(NT):
                xh = io.tile([128, CK, 512], f16, name="xh", tag="x")
                nc.sync.dma_start(out=xh[:], in_=hsv_r[:, :, t * 512 : (t + 1) * 512])
                if t == 0:
                    # deferred const loads: queued behind the tile the first
                    # matmul needs, but well before their own first use
                    nc.sync.dma_start(
                        out=wk_sb[:], in_=wkT.rearrange("(k p) m -> p k m", p=128)
                    )
                    nc.sync.dma_start(
                        out=bk_sb[:], in_=bkd.rearrange("(k p) o -> p k o", p=128)
                    )
                    nc.sync.dma_start(
                        out=wv_sb[:], in_=wvT.rearrange("(k p) m -> p k m", p=128)
                    )
                    nc.sync.dma_start(
                        out=bv_sb[:], in_=bvd.rearrange("(k p) o -> p k o", p=128)
                    )
                    make_identity(nc, ident)
                    nc.sync.dma_start(out=v_sb[:, :, C : C + 2], in_=vone[:])
                for ci in range(CK):
                    ps = pp.tile([128, 512], f32, name="ps_q", tag="pp")
                    for k in range(CK):
                        nc.tensor.matmul(
                            ps,
                            lhsT=wq_sb[:, k, ci * 128 : (ci + 1) * 128],
                            rhs=xh[:, k, :],
                            start=(k == 0),
                            stop=(k == CK - 1),
                        )
                    nc.vector.tensor_scalar_add(
                        q_sb[:, ci, t * 512 : (t + 1) * 512], ps, bq_sb[:, ci, :]
                    )

                xr = io.tile([128, CK, 512], f16, name="xr", tag="x")
                nc.sync.dma_start(out=xr[:], in_=rgb_r[:, :, t * 512 : (t + 1) * 512])
                for ci in range(CK):
                    ps = pp.tile([128, 512], f32, name="ps_k", tag="pp")
                    for k in range(CK):
                        nc.tensor.matmul(
                            ps,
                            lhsT=wk_sb[:, k, ci * 128 : (ci + 1) * 128],
                            rhs=xr[:, k, :],
                            start=(k == 0),
                            stop=(k == CK - 1),
                        )
                    nc.vector.tensor_scalar_add(
                        k_sb[:, ci, t * 512 : (t + 1) * 512], ps, bk_sb[:, ci, :]
                    )

                for jj in range(4):
                    j = t * 4 + jj
                    ps = pp.tile([128, C], f32, name="ps_v", tag="pp")
                    for k in range(CK):
                        nc.tensor.matmul(
                            ps,
                            lhsT=xr[:, k, jj * 128 : (jj + 1) * 128],
                            rhs=wv_sb[:, k, :],
                            start=(k == 0),
                            stop=(k == CK - 1),
                        )
                    nc.vector.tensor_copy(v_sb[:, j, 0:C], ps)

        pt_pool = ctx.enter_context(tc.tile_pool(name="pt", bufs=40))
        spool = ctx.enter_context(tc.tile_pool(name="spsum", bufs=2, space="PSUM"))
        # bufs applies per-tag: 4 tags (po0..po3) x 1 buf = 4 PSUM banks.
        opool = ctx.enter_context(tc.tile_pool(name="opsum", bufs=1, space="PSUM"))
        tpool = ctx.enter_context(tc.tile_pool(name="tpsum", bufs=2, space="PSUM"))
        small = ctx.enter_context(tc.tile_pool(name="small", bufs=6))
        ostage = ctx.enter_context(tc.tile_pool(name="ostage", bufs=4))

        def emit_s(it, j):
            """S^T[j-block, i-tile] = K_j^T Q_i, then P^T = exp(S^T) to SBUF."""
            ps = spool.tile([128, 512], f32, name="ps_s", tag="s")
            for k in range(CK):
                nc.tensor.matmul(
                    ps,
                    lhsT=k_sb[:, k, j * 128 : (j + 1) * 128],
                    rhs=q_sb[:, k, it * 512 : (it + 1) * 512],
                    start=(k == 0),
                    stop=(k == CK - 1),
                )
            pt = pt_pool.tile([128, 512], f16, name="pt", tag="pt")
            nc.scalar.activation(pt, ps, mybir.ActivationFunctionType.Exp)
            return pt

        cur = [emit_s(0, j) for j in range(NJ)]
        for it in range(NT):
            po = [
                opool.tile([128, C + 2], f32, name=f"po{isub}", tag=f"po{isub}")
                for isub in range(NSUB)
            ]
            nxt = [None] * NJ
            for j in range(NJ):
                for isub in range(NSUB):
                    nc.tensor.matmul(
                        po[isub],
                        lhsT=cur[j][:, isub * 128 : (isub + 1) * 128],
                        rhs=v_sb[:, j, :],
                        start=(j == 0),
                        stop=(j == NJ - 1),
                    )
                if it + 1 < NT:
                    nxt[j] = emit_s(it + 1, j)
            for isub in range(NSUB):
                rec = small.tile([128, 1], f32, name="rec", tag="rec")
                nc.vector.reciprocal(rec, po[isub][:, C : C + 1])
                ot = small.tile([128, C], f32, name="ot", tag="ot")
                nc.vector.tensor_scalar_mul(ot, po[isub][:, 0:C], rec)
                for ci in range(CK):
                    tp = tpool.tile([128, 128], f32, name="tp", tag="tp")
                    nc.tensor.transpose(tp, ot[:, ci * 128 : (ci + 1) * 128], ident)
                    ob = ostage.tile([128, 128], f32, name="ob", tag="ob")
                    nc.vector.tensor_scalar_add(ob, tp, bv_sb[:, ci, :])
                    i0 = it * 512 + isub * 128
                    nc.sync.dma_start(
                        out=out[ci * 128 : (ci + 1) * 128, i0 : i0 + 128], in_=ob
                    )
            cur = nxt

    nc.compile()
    return nc


def _get_nc():
    if "nc" not in _CACHE:
        _CACHE["nc"] = _build()
    return _CACHE["nc"]


def kernel(rgb_feat, hsv_feat, Wq, bq, Wk, bk, Wv, bv, _debug=None):
    from concourse.bass_utils import run_bass_kernel_spmd

    rgb_feat = np.ascontiguousarray(np.asarray(rgb_feat, dtype=np.float32)).astype(np.float16)
    hsv_feat = np.ascontiguousarray(np.asarray(hsv_feat, dtype=np.float32)).astype(np.float16)
    scale = np.float32(1.0) / np.sqrt(np.float32(C))
    wqT = np.ascontiguousarray((np.asarray(Wq, np.float32).T * scale).astype(np.float16))
    wkT = np.ascontiguousarray(np.asarray(Wk, np.float32).T.astype(np.float16))
    wvT = np.ascontiguousarray(np.asarray(Wv, np.float32).T.astype(np.float16))
    bq_ = np.ascontiguousarray((np.asarray(bq, np.float32) * scale).reshape(C, 1))
    bk_ = np.ascontiguousarray(np.asarray(bk, np.float32).reshape(C, 1))
    bv_ = np.ascontiguousarray(np.asarray(bv, np.float32).reshape(C, 1))

    in_maps = []
    for bi in range(B):
        in_maps.append(
            {
                "hsv": np.ascontiguousarray(hsv_feat[bi].reshape(C, N)),
                "rgb": np.ascontiguousarray(rgb_feat[bi].reshape(C, N)),
                "wqT": wqT,
                "wkT": wkT,
                "wvT": wvT,
                "bq": bq_,
                "bk": bk_,
                "bv": bv_,
                "vone": _VONE,
            }
        )

    nc = _get_nc()
    kwargs = dict(_debug or {})
    kwargs.pop("result", None)
    res = run_bass_kernel_spmd(nc, in_maps, core_ids=list(range(B)), **kwargs)
    if _debug is not None:
        _debug["result"] = res
    outs = [res.results[bi]["out"].reshape(C, H, W) for bi in range(B)]
    return np.stack(outs, axis=0).astype(np.float32)


# revision 16
# speedup vs baseline: 1.1498x; 1.0697x over previous
"""Cross-attention Bass kernel for Trainium2, data-parallel over batch.

Problem (hardcoded): b=8, c=256, h=w=64 (n=4096).
  q = Wq@hsv + bq; k = Wk@rgb + bk; v = Wv@rgb + bv   (1x1 convs, [c, n])
  attn = softmax_j(q_i . k_j / sqrt(c)); out[c,i] = sum_j v[c,j] attn[i,j]

Per-core design (one batch per NeuronCore, 8 cores):
  - Host pre-transposes weights (WqT/WkT/WvT = W.T), folds the 1/sqrt(c)
    scale into WqT/bq, and converts the matmul data path to fp16 (PSUM
    accumulation stays fp32; measured end-to-end error ~1.7e-4).
  - S^T layout: S^T[j, i] tiles via lhsT=K-chunk, rhs=Q-chunk, so softmax
    axis j lands on PSUM partitions and P^T = exp(S^T) is directly the lhsT
    of the PV matmul. Scores are in [-0.7, 0.7] (tiny weights), so exp
    without max-subtraction is exact softmax.
  - V^T carries a ones column: out^T[i, 0:256] accumulates P@V^T while
    out^T[i, 256] accumulates the softmax denominator in the same matmuls
    (col 257 is zero padding for an even fp16 moving dim).
  - The kernel emits out^T [n, c] (no on-chip transposes at all); the host
    does the final [n,c]->[c,n] transpose and the +bv add (bias passes
    through softmax because attention rows sum to 1).
  - S psum tiles pair two j-blocks [128, 2, 512] so one ACTIVATE exps 1024
    elements, halving ScalarE instruction overhead.
  - Software pipeline: S/exp of i-tile t+1 interleaved with PV of i-tile t;
    the prologue S(0)/exp stream overlaps the Q projection.
"""

import numpy as np

B, C, H, W = 8, 256, 64, 64
N = H * W          # 4096
CK = C // 128      # 2 contraction/channel chunks
NJ = N // 128      # 32 key blocks
NJP = NJ // 2      # 16 paired key blocks
NT = N // 512      # 8 query tiles of 512
NSUB = 4           # 128-wide query sub-blocks per query tile

_CACHE = {}

# ones column for the rowsum trick + zero pad to an even moving dim
_VONE = np.concatenate(
    [np.ones((128, NJ, 1), np.float16), np.zeros((128, NJ, 1), np.float16)], axis=2
)


def _build():
    import concourse.tile as tile
    from concourse import bacc, mybir
    from contextlib import ExitStack

    f32 = mybir.dt.float32
    f16 = mybir.dt.float16

    nc = bacc.Bacc(None, target_bir_lowering=False)

    hsv = nc.dram_tensor("hsv", [C, N], f16, kind="ExternalInput")
    rgb = nc.dram_tensor("rgb", [C, N], f16, kind="ExternalInput")
    wqT = nc.dram_tensor("wqT", [C, C], f16, kind="ExternalInput")
    wkT = nc.dram_tensor("wkT", [C, C], f16, kind="ExternalInput")
    wvT = nc.dram_tensor("wvT", [C, C], f16, kind="ExternalInput")
    bqd = nc.dram_tensor("bq", [C, 1], f32, kind="ExternalInput")
    bkd = nc.dram_tensor("bk", [C, 1], f32, kind="ExternalInput")
    vone = nc.dram_tensor("vone", [128, NJ, 2], f16, kind="ExternalInput")
    # out^T [n, c]: host transposes back and adds bv
    out = nc.dram_tensor("out", [N, C], f32, kind="ExternalOutput")

    with tile.TileContext(nc) as tc, ExitStack() as ctx:
        consts = ctx.enter_context(tc.tile_pool(name="consts", bufs=1))
        big = ctx.enter_context(tc.tile_pool(name="big", bufs=1))

        wq_sb = consts.tile([128, CK, C], f16, name="wq_sb")
        wk_sb = consts.tile([128, CK, C], f16, name="wk_sb")
        wv_sb = consts.tile([128, CK, C], f16, name="wv_sb")
        bq_sb = consts.tile([128, CK, 1], f32, name="bq_sb")
        bk_sb = consts.tile([128, CK, 1], f32, name="bk_sb")

        q_sb = big.tile([128, CK, N], f16, name="q_sb")
        k_sb = big.tile([128, CK, N], f16, name="k_sb")
        v_sb = big.tile([128, NJ, C + 2], f16, name="v_sb")

        hsv_r = hsv.rearrange("(k p) n -> p k n", p=128)
        rgb_r = rgb.rearrange("(k p) n -> p k n", p=128)

        # PSUM budget is 8 banks: spool (2-bank paired tiles x 2 bufs = 4)
        # coexists first with the projection psum pool (4), then with opool
        # (4 tags x 1 buf = 4), which is created only after ppsum closes.
        pt_pool = ctx.enter_context(tc.tile_pool(name="pt", bufs=20))
        spool = ctx.enter_context(tc.tile_pool(name="spsum", bufs=2, space="PSUM"))
        small = ctx.enter_context(tc.tile_pool(name="small", bufs=6))

        def emit_s2(it, jp):
            """S^T for j-blocks (2jp, 2jp+1) x i-tile it, one paired exp."""
            ps = spool.tile([128, 2, 512], f32, name="ps_s", tag="s")
            for b in range(2):
                for k in range(CK):
                    nc.tensor.matmul(
                        ps[:, b, :],
                        lhsT=k_sb[:, k, (2 * jp + b) * 128 : (2 * jp + b + 1) * 128],
                        rhs=q_sb[:, k, it * 512 : (it + 1) * 512],
                        start=(k == 0),
                        stop=(k == CK - 1),
                    )
            pt = pt_pool.tile([128, 2, 512], f16, name="pt", tag="pt")
            nc.scalar.activation(pt, ps, mybir.ActivationFunctionType.Exp)
            return pt

        with (
            tc.tile_pool(name="io", bufs=4) as io,
            tc.tile_pool(name="ppsum", bufs=4, space="PSUM") as pp,
        ):
            # rgb pass: K and V^T projections (first matmul needs wk + xr0)
            nc.sync.dma_start(out=wk_sb[:], in_=wkT.rearrange("(k p) m -> p k m", p=128))
            nc.sync.dma_start(out=bk_sb[:], in_=bkd.rearrange("(k p) o -> p k o", p=128))
            for t in range(NT):
                xr = io.tile([128, CK, 512], f16, name="xr", tag="x")
                nc.sync.dma_start(out=xr[:], in_=rgb_r[:, :, t * 512 : (t + 1) * 512])
                if t == 0:
                    nc.sync.dma_start(
                        out=wv_sb[:], in_=wvT.rearrange("(k p) m -> p k m", p=128)
                    )
                    nc.sync.dma_start(
                        out=wq_sb[:], in_=wqT.rearrange("(k p) m -> p k m", p=128)
                    )
                    nc.sync.dma_start(
                        out=bq_sb[:], in_=bqd.rearrange("(k p) o -> p k o", p=128)
                    )
                    nc.sync.dma_start(out=v_sb[:, :, C : C + 2], in_=vone[:])
                for ci in range(CK):
                    ps = pp.tile([128, 512], f32, name="ps_k", tag="pp")
                    for k in range(CK):
                        nc.tensor.matmul(
                            ps,
                            lhsT=wk_sb[:, k, ci * 128 : (ci + 1) * 128],
                            rhs=xr[:, k, :],
                            start=(k == 0),
                            stop=(k == CK - 1),
                        )
                    nc.vector.tensor_scalar_add(
                        k_sb[:, ci, t * 512 : (t + 1) * 512], ps, bk_sb[:, ci, :]
                    )
                for jj in range(4):
                    j = t * 4 + jj
                    ps = pp.tile([128, C], f32, name="ps_v", tag="pp")
                    for k in range(CK):
                        nc.tensor.matmul(
                            ps,
                            lhsT=xr[:, k, jj * 128 : (jj + 1) * 128],
                            rhs=wv_sb[:, k, :],
                            start=(k == 0),
                            stop=(k == CK - 1),
                        )
                    nc.vector.tensor_copy(v_sb[:, j, 0:C], ps)

            # hsv pass: Q projection; after Q(0), the prologue S(0)/exp
            # stream is interleaved so ScalarE warms up under PE's Q work.
            def emit_q(t, xh):
                for ci in range(CK):
                    ps = pp.tile([128, 512], f32, name="ps_q", tag="pp")
                    for k in range(CK):
                        nc.tensor.matmul(
                            ps,
                            lhsT=wq_sb[:, k, ci * 128 : (ci + 1) * 128],
                            rhs=xh[:, k, :],
                            start=(k == 0),
                            stop=(k == CK - 1),
                        )
                    nc.vector.tensor_scalar_add(
                        q_sb[:, ci, t * 512 : (t + 1) * 512], ps, bq_sb[:, ci, :]
                    )

            xhs = []
            for t in range(NT):
                xh = io.tile([128, CK, 512], f16, name="xh", tag="xh", bufs=8)
                nc.sync.dma_start(out=xh[:], in_=hsv_r[:, :, t * 512 : (t + 1) * 512])
                xhs.append(xh)
            emit_q(0, xhs[0])
            cur = []
            t_next = 1
            for jp in range(NJP):
                cur.append(emit_s2(0, jp))
                if jp % 2 == 1 and t_next < NT:
                    emit_q(t_next, xhs[t_next])
                    t_next += 1

        opool = ctx.enter_context(tc.tile_pool(name="opsum", bufs=1, space="PSUM"))

        for it in range(NT):
            po = [
                opool.tile([128, C + 2], f32, name=f"po{isub}", tag=f"po{isub}")
                for isub in range(NSUB)
            ]
            nxt = [None] * NJP
            for jp in range(NJP):
                for b in range(2):
                    j = 2 * jp + b
                    for isub in range(NSUB):
                        nc.tensor.matmul(
                            po[isub],
                            lhsT=cur[jp][:, b, isub * 128 : (isub + 1) * 128],
                            rhs=v_sb[:, j, :],
                            start=(j == 0),
                            stop=(j == NJ - 1),
                        )
                if it + 1 < NT:
                    nxt[jp] = emit_s2(it + 1, jp)
            for isub in range(NSUB):
                rec = small.tile([128, 1], f32, name="rec", tag="rec")
                nc.vector.reciprocal(rec, po[isub][:, C : C + 1])
                ot = small.tile([128, C], f32, name="ot", tag="ot")
                nc.vector.tensor_scalar_mul(ot, po[isub][:, 0:C], rec)
                i0 = it * 512 + isub * 128
                nc.sync.dma_start(out=out[i0 : i0 + 128, :], in_=ot)
            cur = nxt

    nc.compile()
    return nc


def _get_nc():
    if "nc" not in _CACHE:
        _CACHE["nc"] = _build()
    return _CACHE["nc"]


def kernel(rgb_feat, hsv_feat, Wq, bq, Wk, bk, Wv, bv, _debug=None):
    from concourse.bass_utils import run_bass_kernel_spmd

    rgb16 = np.asarray(rgb_feat, dtype=np.float32).astype(np.float16)
    hsv16 = np.asarray(hsv_feat, dtype=np.float32).astype(np.float16)
    scale = np.float32(1.0) / np.sqrt(np.float32(C))
    wqT = np.ascontiguousarray((np.asarray(Wq, np.float32).T * scale).astype(np.float16))
    wkT = np.ascontiguousarray(np.asarray(Wk, np.float32).T.astype(np.float16))
    wvT = np.ascontiguousarray(np.asarray(Wv, np.float32).T.astype(np.float16))
    bq_ = np.ascontiguousarray((np.asarray(bq, np.float32) * scale).reshape(C, 1))
    bk_ = np.ascontiguousarray(np.asarray(bk, np.float32).reshape(C, 1))
    bv_col = np.asarray(bv, np.float32).reshape(C, 1)

    in_maps = []
    for bi in range(B):
        in_maps.append(
            {
                "hsv": np.ascontiguousarray(hsv16[bi].reshape(C, N)),
                "rgb": np.ascontiguousarray(rgb16[bi].reshape(C, N)),
                "wqT": wqT,
                "wkT": wkT,
                "wvT": wvT,
                "bq": bq_,
                "bk": bk_,
                "vone": _VONE,
            }
        )

    nc = _get_nc()
    kwargs = dict(_debug or {})
    kwargs.pop("result", None)
    res = run_bass_kernel_spmd(nc, in_maps, core_ids=list(range(B)), **kwargs)
    if _debug is not None:
        _debug["result"] = res
    outs = [
        (res.results[bi]["out"].T + bv_col).reshape(C, H, W) for bi in range(B)
    ]
    return np.stack(outs, axis=0).astype(np.float32)
